# revision 1
# baseline (speedup 1.0000x reference)
"""CrossCompress kernel for Trainium2 (Bass/Tile), 8-core data-parallel.

Math: c[b,i,j] = v[b,i]*e[b,j] collapses the einsums to per-row dot products:
    a1 = e . w_vv ; a2 = v . w_ev ; a3 = e . w_ve ; a4 = v . w_ee
    v_out = v*a1 + e*a2 + bias_v
    e_out = v*a3 + e*a4 + bias_e

Per-core plan (shard = 2048 rows; partition p owns DRAM rows [16p, 16p+16),
so every DMA moves 128 contiguous 2KB runs):
  PE  : transpose v,e subtiles -> PSUM; dot matmuls vT.T@[w_ev|w_ee],
        eT.T@[w_vv|w_ve] -> [128,4] PSUM tile of per-row coefficients
  ACT : copy transposed tiles + coefficients PSUM->SBUF
  DVE : v_out = (v*a1 + bias_v_tile) then (e*a2 + prev)   [2 fused STT ops]
  POOL: e_out = (v*a3 + bias_e_tile) then (e*a4 + prev)   [2 fused STT ops]
  DMA : all on SP DGE; loads all dispatched before stores so a store's
        semaphore wait never blocks a later load's dispatch

Host-side packing: the four (128,1) weights travel as one (4,128) tensor
(transposed on-chip by the PE), the two (1,128) biases as one (1,256) row
broadcast to 128 partitions by a stride-0 DMA. 2 setup DMAs total.
"""

from contextlib import ExitStack

import numpy as np

import concourse.bass as bass
import concourse.tile as tile
from concourse import bacc, bass_utils, masks, mybir

B = 16384
D = 128
NCORES = 8
BS = B // NCORES        # 2048 rows per core
P = 128                 # partitions
NT = BS // P            # 16 subtiles per core
CHUNK = 4               # subtiles per DMA chunk
NCH = NT // CHUNK       # 4 chunks
F32 = mybir.dt.float32
# GPSIMD cannot execute TensorScalarPtr on real TRN2 (walrus codegen rejects
# the opcode on Pool), so both output chains run on DVE.
GPSIMD_STT = False


def build_nc():
    nc = bacc.Bacc("TRN2", target_bir_lowering=False, debug=False)

    v_d = nc.dram_tensor("v", [BS, D], F32, kind="ExternalInput").ap()
    e_d = nc.dram_tensor("e", [BS, D], F32, kind="ExternalInput").ap()
    # rows: [w_ev, w_ee, w_vv, w_ve]
    wpack = nc.dram_tensor("wpack", [4, D], F32, kind="ExternalInput").ap()
    # cols: [bias_v | bias_e]
    bpack = nc.dram_tensor("bpack", [1, 2 * D], F32, kind="ExternalInput").ap()
    vo_d = nc.dram_tensor("v_out", [BS, D], F32, kind="ExternalOutput").ap()
    eo_d = nc.dram_tensor("e_out", [BS, D], F32, kind="ExternalOutput").ap()

    # row (p*NT + n) lives at partition p, free-slot n -> per-partition data
    # is one contiguous run in DRAM (NT*D*4 = 8KB)
    v_r = v_d.rearrange("(p n) d -> p n d", p=P)
    e_r = e_d.rearrange("(p n) d -> p n d", p=P)
    vo_r = vo_d.rearrange("(p n) d -> p n d", p=P)
    eo_r = eo_d.rearrange("(p n) d -> p n d", p=P)

    mult = mybir.AluOpType.mult
    add = mybir.AluOpType.add

    with tile.TileContext(nc) as tc, ExitStack() as ctx:
        const = ctx.enter_context(tc.tile_pool(name="const", bufs=1))

        identity = const.tile([P, P], F32)
        masks.make_identity(nc, identity[:])

        # --- setup + input loads, dispatched up front (SP order matters:
        # wpack first so w4 derives early, first data chunk next, bias row
        # after it, then the remaining loads; stores are emitted later so
        # their semaphore waits never sit ahead of a load) ---
        vpool = ctx.enter_context(tc.tile_pool(name="vin", bufs=NCH))
        epool = ctx.enter_context(tc.tile_pool(name="ein", bufs=NCH))
        wrows = const.tile([4, D], F32)
        bcast = const.tile([P, 2 * D], F32)
        v_chs, e_chs = [], []
        for c in range(NCH):
            csl = slice(c * CHUNK, (c + 1) * CHUNK)
            v_ch = vpool.tile([P, CHUNK * D], F32)
            e_ch = epool.tile([P, CHUNK * D], F32)
            nc.sync.dma_start(
                v_ch[:].rearrange("p (n d) -> p n d", d=D), v_r[:, csl, :]
            )
            nc.sync.dma_start(
                e_ch[:].rearrange("p (n d) -> p n d", d=D), e_r[:, csl, :]
            )
            v_chs.append(v_ch)
            e_chs.append(e_ch)
            if c == 0:
                # consts land right after the first data chunk: the first
                # dot-matmul/STT needs them ~1.5us after the first transpose
                nc.sync.dma_start(wrows[:], wpack)
                nc.sync.dma_start(bcast[:], bpack.broadcast_to((P, 2 * D)))
        bv_t = bcast[:, 0:D]
        be_t = bcast[:, D : 2 * D]

        # weights transposed on-chip: [4,128] -> PSUM [128,4] -> SBUF
        # w4 cols: [w_ev, w_ee, w_vv, w_ve]
        psw_pool = ctx.enter_context(tc.tile_pool(name="psW", bufs=1, space="PSUM"))
        psw = psw_pool.tile([P, 4], F32)
        nc.tensor.transpose(psw[:], wrows[:], identity[0:4, 0:4])
        w4 = const.tile([P, 4], F32)
        nc.scalar.copy(w4[:], psw[:])

        vopool = ctx.enter_context(tc.tile_pool(name="vo", bufs=1))
        eopool = ctx.enter_context(tc.tile_pool(name="eo", bufs=1))
        tpool = ctx.enter_context(tc.tile_pool(name="tmp", bufs=3))
        sbt_pool = ctx.enter_context(tc.tile_pool(name="sbT", bufs=3))
        dsb_pool = ctx.enter_context(tc.tile_pool(name="dots", bufs=4))
        pst_pool = ctx.enter_context(tc.tile_pool(name="psT", bufs=3, space="PSUM"))
        psd_pool = ctx.enter_context(tc.tile_pool(name="psD", bufs=4, space="PSUM"))

        # one-stage software pipeline over subtiles: stage A/B (transpose +
        # te-copy) for subtile k is emitted before stage C/D/E (dots + out
        # assembly) of subtile k-1, so the PE sequencer is never parked on a
        # wait for te(k) while transpose(k+1) could already dispatch.
        OC = CHUNK  # subtiles per output DMA
        NOC = NT // OC
        pst_tiles = [None] * NT
        te_tiles = [None] * NT
        vo_chs = [
            vopool.tile([P, OC * D], F32, name=f"voc{i}", tag=f"voc{i}")
            for i in range(NOC)
        ]
        eo_chs = [
            eopool.tile([P, OC * D], F32, name=f"eoc{i}", tag=f"eoc{i}")
            for i in range(NOC)
        ]

        def in_sl(t, k):
            return t[k // CHUNK][:, bass.ts(k % CHUNK, D)]

        def stage_ab(k):
            v_sl, e_sl = in_sl(v_chs, k), in_sl(e_chs, k)
            pst = pst_pool.tile([P, 2 * P], F32, tag="pst")
            nc.tensor.transpose(pst[:, 0:P], v_sl, identity[:])
            nc.tensor.transpose(pst[:, P : 2 * P], e_sl, identity[:])
            te = sbt_pool.tile([P, 2 * P], F32, tag="te")
            nc.scalar.copy(te[:], pst[:])
            pst_tiles[k], te_tiles[k] = pst, te

        def stage_cde(k):
            v_sl, e_sl = in_sl(v_chs, k), in_sl(e_chs, k)
            te = te_tiles[k]
            vo_sl = vo_chs[k // OC][:, bass.ts(k % OC, D)]
            eo_sl = eo_chs[k // OC][:, bass.ts(k % OC, D)]

            # dots: psd cols = [a2=v.w_ev, a4=v.w_ee, a1=e.w_vv, a3=e.w_ve]
            psd = psd_pool.tile([P, 4], F32, tag="psd")
            nc.tensor.matmul(
                psd[:, 0:2], lhsT=te[:, 0:P], rhs=w4[:, 0:2],
                start=True, stop=False,
            )
            nc.tensor.matmul(
                psd[:, 2:4], lhsT=te[:, P : 2 * P], rhs=w4[:, 2:4],
                start=False, stop=True,
            )

            # one chain on DVE, one on GPSIMD, alternating per subtile so
            # both engines finish together. POOL can't read PSUM, so the
            # coefficients hop through SBUF via a small DVE copy dispatched
            # first (it unblocks POOL before the DVE STTs run).
            if GPSIMD_STT:
                dsb = dsb_pool.tile([P, 4], F32, tag="dsb")
                nc.vector.tensor_copy(dsb[:], psd[:])
                pool_eng = nc.gpsimd
                a_pool = dsb
            else:
                pool_eng = nc.vector
                a_pool = psd
            # chain specs: (engine, coef_src, scale_col, add_col, bias, out)
            # v_out = (v*a1 + bias_v) ; += e*a2   (cols: a1=2, a2=0)
            # e_out = (v*a3 + bias_e) ; += e*a4   (cols: a3=3, a4=1)
            chains = [
                (nc.vector, psd, 2, 0, bv_t, vo_sl, "tmpv"),
                (pool_eng, a_pool, 3, 1, be_t, eo_sl, "tmpe"),
            ]
            if k % 2:
                chains = [
                    (nc.vector, psd, 3, 1, be_t, eo_sl, "tmpe"),
                    (pool_eng, a_pool, 2, 0, bv_t, vo_sl, "tmpv"),
                ]
            for eng, coef, c0, c1, bias_t, out_sl, ttag in chains:
                tmp = tpool.tile([P, D], F32, name=ttag, tag=ttag)
                eng.scalar_tensor_tensor(
                    out=tmp[:], in0=v_sl, scalar=coef[:, c0 : c0 + 1],
                    in1=bias_t, op0=mult, op1=add,
                )
                eng.scalar_tensor_tensor(
                    out=out_sl, in0=e_sl, scalar=coef[:, c1 : c1 + 1],
                    in1=tmp[:], op0=mult, op1=add,
                )

            if (k + 1) % OC == 0:
                h = k // OC
                osl = slice(h * OC, (h + 1) * OC)
                nc.sync.dma_start(
                    vo_r[:, osl, :],
                    vo_chs[h][:].rearrange("p (n d) -> p n d", d=D),
                )
                nc.sync.dma_start(
                    eo_r[:, osl, :],
                    eo_chs[h][:].rearrange("p (n d) -> p n d", d=D),
                )

        for k in range(NT + 1):
            if k < NT:
                stage_ab(k)
            if k >= 1:
                stage_cde(k - 1)

    nc.finalize()
    return nc


_NC_CACHE = {}


def _get_nc():
    if "nc" not in _NC_CACHE:
        _NC_CACHE["nc"] = build_nc()
    return _NC_CACHE["nc"]


def make_in_maps(inputs):
    v = np.ascontiguousarray(inputs["v"], dtype=np.float32)
    e = np.ascontiguousarray(inputs["e"], dtype=np.float32)
    wpack = np.ascontiguousarray(
        np.stack(
            [
                np.asarray(inputs["weight_ev"], dtype=np.float32).reshape(D),
                np.asarray(inputs["weight_ee"], dtype=np.float32).reshape(D),
                np.asarray(inputs["weight_vv"], dtype=np.float32).reshape(D),
                np.asarray(inputs["weight_ve"], dtype=np.float32).reshape(D),
            ]
        )
    )
    bpack = np.ascontiguousarray(
        np.concatenate(
            [
                np.asarray(inputs["bias_v"], dtype=np.float32).reshape(1, D),
                np.asarray(inputs["bias_e"], dtype=np.float32).reshape(1, D),
            ],
            axis=1,
        )
    )
    in_maps = []
    for i in range(NCORES):
        in_maps.append(
            {
                "v": np.ascontiguousarray(v[i * BS : (i + 1) * BS]),
                "e": np.ascontiguousarray(e[i * BS : (i + 1) * BS]),
                "wpack": wpack,
                "bpack": bpack,
            }
        )
    return in_maps


def run_spmd(inputs, **kwargs):
    nc = _get_nc()
    return bass_utils.run_bass_kernel_spmd(
        nc, make_in_maps(inputs), core_ids=list(range(NCORES)), **kwargs
    )


def kernel(**inputs):
    res = run_spmd(inputs)
    v_out = np.concatenate([r["v_out"] for r in res.results], axis=0)
    e_out = np.concatenate([r["e_out"] for r in res.results], axis=0)
    return (v_out, e_out)


if __name__ == "__main__":
    rng = np.random.default_rng(0)
    demo = {
        "v": rng.standard_normal((B, D), dtype=np.float32),
        "e": rng.standard_normal((B, D), dtype=np.float32),
        "weight_vv": rng.standard_normal((D, 1)).astype(np.float32) * 0.2,
        "weight_ev": rng.standard_normal((D, 1)).astype(np.float32) * 0.2,
        "weight_ve": rng.standard_normal((D, 1)).astype(np.float32) * 0.2,
        "weight_ee": rng.standard_normal((D, 1)).astype(np.float32) * 0.2,
        "bias_v": rng.standard_normal((1, D)).astype(np.float32) * 0.2,
        "bias_e": rng.standard_normal((1, D)).astype(np.float32) * 0.2,
    }
    vo, eo = kernel(**demo)
    a1 = demo["e"] @ demo["weight_vv"]
    a2 = demo["v"] @ demo["weight_ev"]
    a3 = demo["e"] @ demo["weight_ve"]
    a4 = demo["v"] @ demo["weight_ee"]
    vo_ref = demo["v"] * a1 + demo["e"] * a2 + demo["bias_v"]
    eo_ref = demo["v"] * a3 + demo["e"] * a4 + demo["bias_e"]
    for name, got, ref in (("v_out", vo, vo_ref), ("e_out", eo, eo_ref)):
        err = np.abs(got - ref).max() / max(np.abs(ref).max(), 1e-9)
        print(f"{name}: rel abs err = {err:.3e}")

